# revision 4
# baseline (speedup 1.0000x reference)
import numpy as np

B, N, D = 32, 128, 512
DIR = 2
L = 16
NC = 8
BPC = B // NC
BN = BPC * N
P = 128
KC = D // P
N_WARM = 20

_prog_cache: dict = {}


def _build():
    import concourse.bass as bass
    import concourse.mybir as mybir
    import concourse.tile as tile
    from concourse import bacc

    f32 = mybir.dt.float32
    bf16 = mybir.dt.bfloat16

    nc = bacc.Bacc(
        "TRN2",
        target_bir_lowering=False,
        debug=False,
        num_devices=NC,
    )

    gT_d = nc.dram_tensor("gT", [P, BN], bf16, kind="ExternalInput").ap()
    FT_d = nc.dram_tensor("FT", [P, KC, BN], bf16, kind="ExternalInput").ap()
    WT_d = nc.dram_tensor("WT", [P, KC, D], bf16, kind="ExternalInput").ap()
    esel_d = nc.dram_tensor("esel", [P, L * L], bf16, kind="ExternalInput").ap()
    bias_d = nc.dram_tensor("bias", [L, D], bf16, kind="ExternalInput").ap()
    out = nc.dram_tensor("out", [BPC, N, D], f32, kind="ExternalOutput").ap()

    with tile.TileContext(nc) as tc:
        with (
            tc.tile_pool(name="work", bufs=1) as wpool,
            tc.tile_pool(name="psum", bufs=1, space="PSUM") as ppool,
        ):
            gT = wpool.tile([P, BN], bf16)
            nc.sync.dma_start(out=gT, in_=gT_d)

            esel = wpool.tile([P, L * L], bf16)
            nc.scalar.dma_start(out=esel, in_=esel_d)

            FT = wpool.tile([P, KC, BN], bf16)
            nc.sync.dma_start(out=FT, in_=FT_d)

            bias_sb = wpool.tile([L, D], bf16)
            nc.scalar.dma_start(out=bias_sb, in_=bias_d)

            WT = wpool.tile([P, KC, D], bf16)
            nc.scalar.dma_start(out=WT, in_=WT_d)

            warm = wpool.tile([P, P], bf16)
            nc.gpsimd.memset(warm, 0.0)
            psum_warm = ppool.tile([P, 64], f32, tag="warm", bufs=1)
            for _ in range(N_WARM):
                nc.tensor.matmul(
                    out=psum_warm,
                    lhsT=warm,
                    rhs=warm[:, 0:64],
                    start=True,
                    stop=True,
                )

            act_warm = wpool.tile([P, 2], f32)
            nc.scalar.copy(out=act_warm[:, 0:1], in_=warm[:, 0:1])

            EQ = wpool.tile([P, L, BN], bf16)
            psum_cnt = ppool.tile([L, BN], f32, tag="cnt", bufs=1)
            for l in range(L):
                eng = nc.gpsimd if l % 3 == 2 else nc.vector
                eng.tensor_scalar(
                    out=EQ[:, l, :],
                    in0=gT,
                    scalar1=float(l),
                    scalar2=None,
                    op0=mybir.AluOpType.is_equal,
                )
                nc.tensor.matmul(
                    out=psum_cnt,
                    lhsT=esel[:, l * L : (l + 1) * L],
                    rhs=EQ[:, l, :],
                    start=(l == 0),
                    stop=(l == L - 1),
                )
            cntT = wpool.tile([L, BN], bf16)
            nc.scalar.copy(out=cntT, in_=psum_cnt)

            out_sb = wpool.tile([P, BPC, D], f32)
            for b in range(BPC):
                psum_out = ppool.tile([P, D], f32, tag="out", bufs=3)
                sl = slice(b * P, (b + 1) * P)
                for c in range(KC):
                    nc.tensor.matmul(
                        out=psum_out,
                        lhsT=FT[:, c, sl],
                        rhs=WT[:, c, :],
                        start=(c == 0),
                        stop=False,
                    )
                nc.tensor.matmul(
                    out=psum_out,
                    lhsT=cntT[:, sl],
                    rhs=bias_sb,
                    start=False,
                    stop=True,
                )
                if b == BPC - 1:
                    h = D // 2
                    nc.vector.tensor_copy(out=out_sb[:, b, 0:h], in_=psum_out[:, 0:h])
                    nc.scalar.copy(out=out_sb[:, b, h:D], in_=psum_out[:, h:D])
                    nc.sync.dma_start(out=out[b, :, 0:h], in_=out_sb[:, b, 0:h])
                    nc.scalar.dma_start(out=out[b, :, h:D], in_=out_sb[:, b, h:D])
                else:
                    if b % 2 == 0:
                        nc.vector.tensor_copy(out=out_sb[:, b, :], in_=psum_out)
                    else:
                        nc.scalar.copy(out=out_sb[:, b, :], in_=psum_out)
                    ring = nc.sync if b % 2 == 0 else nc.scalar
                    ring.dma_start(out=out[b], in_=out_sb[:, b, :])

    nc.compile()
    return nc


def _get_prog():
    if "p" not in _prog_cache:
        _prog_cache["p"] = _build()
    return _prog_cache["p"]


def _shard_inputs(feature, graph, weights, bias):
    import ml_dtypes

    bf16 = ml_dtypes.bfloat16

    feature = np.asarray(feature, dtype=np.float32)
    weights = np.asarray(weights, dtype=np.float32)
    bias = np.asarray(bias, dtype=np.float32)
    g = np.asarray(graph)
    if g.dtype == np.int64:
        g32 = g.view(np.int32)[..., ::2]
    else:
        g32 = g.astype(np.int32)

    M = weights.sum(axis=0) + np.eye(D, dtype=np.float32)
    WT_h = np.ascontiguousarray(
        M.T.reshape(KC, P, D).transpose(1, 0, 2).astype(bf16)
    )

    esel_h = np.zeros((P, L, L), dtype=bf16)
    idx = np.arange(L)
    esel_h[:, idx, idx] = 1.0
    esel_h = esel_h.reshape(P, L * L)

    bias_h = bias.astype(bf16)

    in_maps = []
    for core in range(NC):
        sl = slice(core * BPC, (core + 1) * BPC)
        Fc = feature[sl].reshape(BN, D)
        FT_h = np.ascontiguousarray(
            Fc.T.reshape(KC, P, BN).transpose(1, 0, 2).astype(bf16)
        )
        gc = g32[sl].reshape(BN, N)
        gT_h = np.ascontiguousarray(gc.T.astype(bf16))
        in_maps.append(
            {
                "gT": gT_h,
                "FT": FT_h,
                "WT": WT_h,
                "esel": esel_h,
                "bias": bias_h,
            }
        )
    return in_maps


def _run(feature, graph, weights, bias, trace=False):
    from concourse.bass_utils import run_bass_kernel_spmd

    in_maps = _shard_inputs(feature, graph, weights, bias)
    nc = _get_prog()
    res = run_bass_kernel_spmd(nc, in_maps, core_ids=list(range(NC)), trace=trace)
    out = np.concatenate([r["out"] for r in res.results], axis=0)
    return out, res


def kernel(feature, graph, weights, bias):
    out, _ = _run(feature, graph, weights, bias, trace=False)
    return out


# revision 6
# speedup vs baseline: 2.2300x; 2.2300x over previous
import numpy as np

B, N, D = 32, 128, 512
DIR = 2
L = 16
LH = 15
NC = 8
BPC = B // NC
BN = BPC * N
P = 128
KC = D // P
N_WARM = 18

_prog_cache: dict = {}


def _build():
    import concourse.bass as bass
    import concourse.mybir as mybir
    import concourse.tile as tile
    from concourse import bacc

    f32 = mybir.dt.float32
    bf16 = mybir.dt.bfloat16

    nc = bacc.Bacc(
        "TRN2",
        target_bir_lowering=False,
        debug=False,
        num_devices=NC,
    )

    gT_d = nc.dram_tensor("gT", [P, BN], bf16, kind="ExternalInput").ap()
    FT_d = nc.dram_tensor("FT", [P, KC, BN], bf16, kind="ExternalInput").ap()
    WT_d = nc.dram_tensor("WT", [P, KC, D], bf16, kind="ExternalInput").ap()
    esel_d = nc.dram_tensor("esel", [P, LH * LH], bf16, kind="ExternalInput").ap()
    bias_d = nc.dram_tensor("bias2", [L, D], bf16, kind="ExternalInput").ap()
    out = nc.dram_tensor("out", [BPC, N, D], bf16, kind="ExternalOutput").ap()

    with tile.TileContext(nc) as tc:
        with (
            tc.tile_pool(name="work", bufs=1) as wpool,
            tc.tile_pool(name="psum", bufs=1, space="PSUM") as ppool,
        ):
            gT = wpool.tile([P, BN], bf16)
            nc.scalar.dma_start(out=gT, in_=gT_d)

            FT = wpool.tile([P, KC, BN], bf16)
            nc.sync.dma_start(out=FT, in_=FT_d)

            esel = wpool.tile([P, LH * LH], bf16)
            nc.scalar.dma_start(out=esel, in_=esel_d)

            bias_sb = wpool.tile([L, D], bf16)
            nc.scalar.dma_start(out=bias_sb, in_=bias_d)

            WT = wpool.tile([P, KC, D], bf16)
            nc.scalar.dma_start(out=WT, in_=WT_d)

            warm = wpool.tile([P, P], bf16)
            nc.gpsimd.memset(warm, 0.0)
            psum_warm = ppool.tile([P, 64], f32, tag="warm", bufs=1)
            for _ in range(N_WARM):
                nc.tensor.matmul(
                    out=psum_warm,
                    lhsT=warm,
                    rhs=warm[:, 0:64],
                    start=True,
                    stop=True,
                )

            act_warm = wpool.tile([P, 2], f32)
            nc.scalar.copy(out=act_warm[:, 0:1], in_=warm[:, 0:1])

            cntT = wpool.tile([L, BN], bf16)
            nc.vector.memset(cntT, 1.0)

            EQ = wpool.tile([P, LH, BN], bf16)
            psum_cnt = ppool.tile([LH, BN], f32, tag="cnt", bufs=1)

            def emit_eq(l):
                nc.vector.tensor_scalar(
                    out=EQ[:, l, :],
                    in0=gT,
                    scalar1=float(l),
                    scalar2=None,
                    op0=mybir.AluOpType.is_equal,
                )

            def emit_cnt(l):
                nc.tensor.matmul(
                    out=psum_cnt,
                    lhsT=esel[:, l * LH : (l + 1) * LH],
                    rhs=EQ[:, l, :],
                    start=(l == 0),
                    stop=(l == LH - 1),
                )

            psum_outs = [
                ppool.tile([P, D], f32, tag="out", bufs=BPC, name=f"psum_out{b}")
                for b in range(BPC)
            ]

            def emit_main(i):
                b, c = divmod(i, KC)
                nc.tensor.matmul(
                    out=psum_outs[b],
                    lhsT=FT[:, c, b * P : (b + 1) * P],
                    rhs=WT[:, c, :],
                    start=(c == 0),
                    stop=False,
                )

            for l in range(3):
                emit_eq(l)
                emit_cnt(l)
            mi = 0
            for l in range(3, LH):
                emit_eq(l)
                emit_main(mi)
                mi += 1
                emit_cnt(l)
            while mi < BPC * KC:
                emit_main(mi)
                mi += 1

            nc.scalar.copy(out=cntT[0:LH, :], in_=psum_cnt)

            out_sb = wpool.tile([P, BPC, D], bf16)
            for b in range(BPC):
                sl = slice(b * P, (b + 1) * P)
                nc.tensor.matmul(
                    out=psum_outs[b],
                    lhsT=cntT[:, sl],
                    rhs=bias_sb,
                    start=False,
                    stop=True,
                )
                if b == BPC - 1:
                    h = D // 2
                    nc.vector.tensor_copy(out=out_sb[:, b, 0:h], in_=psum_outs[b][:, 0:h])
                    nc.scalar.copy(out=out_sb[:, b, h:D], in_=psum_outs[b][:, h:D])
                    nc.sync.dma_start(out=out[b, :, 0:h], in_=out_sb[:, b, 0:h])
                    nc.scalar.dma_start(out=out[b, :, h:D], in_=out_sb[:, b, h:D])
                else:
                    if b % 2 == 0:
                        nc.vector.tensor_copy(out=out_sb[:, b, :], in_=psum_outs[b])
                    else:
                        nc.scalar.copy(out=out_sb[:, b, :], in_=psum_outs[b])
                    ring = nc.sync if b % 2 == 0 else nc.scalar
                    ring.dma_start(out=out[b], in_=out_sb[:, b, :])

    nc.compile()
    return nc


def _get_prog():
    if "p" not in _prog_cache:
        _prog_cache["p"] = _build()
    return _prog_cache["p"]


def _shard_inputs(feature, graph, weights, bias):
    import ml_dtypes

    bf16 = ml_dtypes.bfloat16

    feature = np.asarray(feature, dtype=np.float32)
    weights = np.asarray(weights, dtype=np.float32)
    bias = np.asarray(bias, dtype=np.float32)
    g = np.asarray(graph)
    if g.dtype == np.int64:
        g32 = g.view(np.int32)[..., ::2]
    else:
        g32 = g.astype(np.int32)

    M = weights.sum(axis=0) + np.eye(D, dtype=np.float32)
    WT_h = np.ascontiguousarray(M.T.reshape(KC, P, D).transpose(1, 0, 2).astype(bf16))

    esel_h = np.zeros((P, LH, LH), dtype=bf16)
    idx = np.arange(LH)
    esel_h[:, idx, idx] = 1.0
    esel_h = esel_h.reshape(P, LH * LH)

    bias2 = bias - bias[L - 1]
    bias2[L - 1] = N * bias[L - 1]
    bias_h = bias2.astype(bf16)

    in_maps = []
    for core in range(NC):
        sl = slice(core * BPC, (core + 1) * BPC)
        Fc = feature[sl].reshape(BN, D)
        FT_h = np.ascontiguousarray(Fc.T.reshape(KC, P, BN).transpose(1, 0, 2).astype(bf16))
        gc = g32[sl].reshape(BN, N)
        gT_h = np.ascontiguousarray(gc.T.astype(bf16))
        in_maps.append(
            {"gT": gT_h, "FT": FT_h, "WT": WT_h, "esel": esel_h, "bias2": bias_h}
        )
    return in_maps


def _run(feature, graph, weights, bias, trace=False):
    from concourse.bass_utils import run_bass_kernel_spmd

    in_maps = _shard_inputs(feature, graph, weights, bias)
    nc = _get_prog()
    res = run_bass_kernel_spmd(nc, in_maps, core_ids=list(range(NC)), trace=trace)
    out = np.concatenate(
        [np.asarray(r["out"]).astype(np.float32) for r in res.results], axis=0
    )
    return out, res


def kernel(feature, graph, weights, bias):
    out, _ = _run(feature, graph, weights, bias, trace=False)
    return out
